# revision 2
# baseline (speedup 1.0000x reference)
"""Trainium2 Bass kernel for nn_DenseRelativeLoc.

Data-parallel over batch: 32 images per NeuronCore x 8 cores.

Per image b (all activations kept transposed, feature-dim on partitions):
    z1a = x_flat[b]^T @ W1[:768]    [196hw, 512]   (natural layouts, no transposes)
    z1b = x_flat[b]^T @ W1[768:]    [196hw, 512]
    h1T[m, s] = relu( sum_hw z1a[hw, m]*onehot_x[hw, s]
                    + sum_hw z1b[hw, m]*onehot_y[hw, s] + b1[m] )
    (the gather of sampled points is folded into a one-hot matmul; this is
     exact because onehot selects the hw = idx[s] row)
    h2T = relu(W2^T @ h1T + b2)     [512, 196]
    predT = W3^T @ h2T + b3         [2, 196]
All matmuls in bf16 with fp32 PSUM accumulation.
deltaxy = float(pxs - pys) + 13 computed on-device with DVE ops.
"""
import sys
import types

import numpy as np
import ml_dtypes

B, C, H, W_IMG = 256, 768, 14, 14
S = 196          # sample count == H*W
HID = 512
OUT = 2
N_CORES = 8
BPC = B // N_CORES      # batches per core
KC = C // 128           # 6 contraction chunks for layer 1
MJ = HID // 128         # 4 HID chunks
HW_CHUNKS = [(0, 128), (128, 68)]   # hw/sample partition chunks of 196


def _install_ntff_hook():
    try:
        import antenv.axon_hooks  # noqa: F401
        return
    except ImportError:
        pass
    try:
        from trn_agent_boot.trn_boot import _ntff_profile_via_ctypes
        hook = _ntff_profile_via_ctypes("/opt/axon/libaxon_pjrt.so")
    except Exception:
        hook = None
    mod = types.ModuleType("antenv.axon_hooks")
    mod.get_axon_ntff_profile_hook = lambda: hook
    sys.modules["antenv.axon_hooks"] = mod


def _build_nc():
    from contextlib import ExitStack

    import concourse.bass as bass
    import concourse.bacc as bacc
    import concourse.mybir as mybir
    import concourse.tile as tile

    dt = mybir.dt
    f32, bf16, i32 = dt.float32, dt.bfloat16, dt.int32
    AF = mybir.ActivationFunctionType
    ALU = mybir.AluOpType

    nc = bacc.Bacc(None, target_bir_lowering=False)

    x_t = nc.dram_tensor("x", [BPC, C, S], f32, kind="ExternalInput")
    pxs_t = nc.dram_tensor("pxs", [BPC, 2 * S], i32, kind="ExternalInput")
    pys_t = nc.dram_tensor("pys", [BPC, 2 * S], i32, kind="ExternalInput")
    w1_t = nc.dram_tensor("W1", [2 * C, HID], f32, kind="ExternalInput")
    w2_t = nc.dram_tensor("W2", [HID, HID], f32, kind="ExternalInput")
    w3_t = nc.dram_tensor("W3", [HID, OUT], f32, kind="ExternalInput")
    b1_t = nc.dram_tensor("b1", [HID], f32, kind="ExternalInput")
    b2_t = nc.dram_tensor("b2", [HID], f32, kind="ExternalInput")
    b3_t = nc.dram_tensor("b3", [OUT], f32, kind="ExternalInput")
    iota_t = nc.dram_tensor("iota196", [S], f32, kind="ExternalInput")
    pred_t = nc.dram_tensor("predxy", [BPC * S, OUT], f32, kind="ExternalOutput")
    delta_t = nc.dram_tensor("deltaxy", [BPC * S, OUT], f32, kind="ExternalOutput")

    with ExitStack() as ctx:
        tc = ctx.enter_context(tile.TileContext(nc))
        wpool = ctx.enter_context(tc.tile_pool(name="w", bufs=1))
        wstage = ctx.enter_context(tc.tile_pool(name="wstage", bufs=2))
        xpool = ctx.enter_context(tc.tile_pool(name="x", bufs=3))
        xbfpool = ctx.enter_context(tc.tile_pool(name="xbf", bufs=2))
        zsb = ctx.enter_context(tc.tile_pool(name="zsb", bufs=2))
        ohpool = ctx.enter_context(tc.tile_pool(name="oh", bufs=2))
        bcpool = ctx.enter_context(tc.tile_pool(name="bc", bufs=2))
        h1pool = ctx.enter_context(tc.tile_pool(name="h1", bufs=2))
        h2pool = ctx.enter_context(tc.tile_pool(name="h2", bufs=2))
        opool = ctx.enter_context(tc.tile_pool(name="op", bufs=4))
        idxpool = ctx.enter_context(tc.tile_pool(name="idx", bufs=1))
        zps = ctx.enter_context(tc.tile_pool(name="zps", bufs=1, space="PSUM"))
        hps = ctx.enter_context(tc.tile_pool(name="hps", bufs=1, space="PSUM"))
        dram = ctx.enter_context(tc.tile_pool(name="dram", bufs=1, space="DRAM"))

        # ---------------- weights: load fp32, cast to bf16 on-chip ----------
        w1b = []
        for k in range(2 * KC):
            st = wstage.tile([128, HID], f32, name=f"w1st{k}", tag="wst")
            nc.sync.dma_start(st[:], w1_t[k * 128:(k + 1) * 128, :])
            wb = wpool.tile([128, HID], bf16, name=f"w1b{k}", tag=f"w1b{k}")
            nc.vector.tensor_copy(wb[:], st[:])
            w1b.append(wb)
        w2b = []
        for k in range(MJ):
            st = wstage.tile([128, HID], f32, name=f"w2st{k}", tag="wst")
            nc.sync.dma_start(st[:], w2_t[k * 128:(k + 1) * 128, :])
            wb = wpool.tile([128, HID], bf16, name=f"w2b{k}", tag=f"w2b{k}")
            nc.vector.tensor_copy(wb[:], st[:])
            w2b.append(wb)
        w3b = []
        for k in range(MJ):
            st = wstage.tile([128, OUT], f32, name=f"w3st{k}", tag="w3st")
            nc.sync.dma_start(st[:], w3_t[k * 128:(k + 1) * 128, :])
            wb = wpool.tile([128, OUT], bf16, name=f"w3b{k}", tag=f"w3b{k}")
            nc.vector.tensor_copy(wb[:], st[:])
            w3b.append(wb)
        b1c, b2c = [], []
        for j in range(MJ):
            t1 = wpool.tile([128, 1], f32, name=f"b1c{j}", tag=f"b1c{j}")
            nc.sync.dma_start(t1[:], b1_t[j * 128:(j + 1) * 128])
            b1c.append(t1)
            t2 = wpool.tile([128, 1], f32, name=f"b2c{j}", tag=f"b2c{j}")
            nc.sync.dma_start(t2[:], b2_t[j * 128:(j + 1) * 128])
            b2c.append(t2)
        b3c = wpool.tile([OUT, 1], f32, name="b3c", tag="b3c")
        nc.sync.dma_start(b3c[:], b3_t[:])
        iota_lo = wpool.tile([128, 1], f32, name="iota_lo", tag="iota_lo")
        nc.sync.dma_start(iota_lo[:], iota_t[0:128])
        iota_hi = wpool.tile([68, 1], f32, name="iota_hi", tag="iota_hi")
        nc.sync.dma_start(iota_hi[:], iota_t[128:S])

        # ---------------- per-core index prep + deltaxy ---------------------
        pxs_sb = idxpool.tile([BPC, 2 * S], i32, name="pxs_sb", tag="pxs_sb")
        nc.sync.dma_start(pxs_sb[:], pxs_t[:, :])
        pys_sb = idxpool.tile([BPC, 2 * S], i32, name="pys_sb", tag="pys_sb")
        nc.sync.dma_start(pys_sb[:], pys_t[:, :])

        dsub = idxpool.tile([BPC, 2 * S], i32, name="dsub", tag="dsub")
        nc.vector.tensor_tensor(dsub[:], pxs_sb[:], pys_sb[:], ALU.subtract)
        ddel = idxpool.tile([BPC, 2 * S], f32, name="ddel", tag="ddel")
        nc.vector.tensor_scalar(ddel[:], dsub[:], float(H - 1), None, op0=ALU.add)
        nc.sync.dma_start(
            bass.AP(delta_t, 0, [[2 * S, BPC], [1, 2 * S]]), ddel[:]
        )

        # idx = px_row*14 + px_col, as bf16 (exact: values < 256)
        idx_dram = {}
        for nm, src in (("x", pxs_sb), ("y", pys_sb)):
            tmp = idxpool.tile([BPC, S], i32, name=f"itmp{nm}", tag=f"itmp{nm}")
            nc.vector.tensor_scalar(
                tmp[:], src[:, 0:2 * S:2], float(W_IMG), None, op0=ALU.mult
            )
            ib = idxpool.tile([BPC, S], bf16, name=f"ibf{nm}", tag=f"ibf{nm}")
            nc.vector.tensor_tensor(ib[:], tmp[:], src[:, 1:2 * S:2], ALU.add)
            d = dram.tile([BPC, S], bf16, name=f"idxd{nm}", tag=f"idxd{nm}")
            nc.sync.dma_start(d[:, :], ib[:])
            idx_dram[nm] = d

        # ---------------- per-batch emission helpers ------------------------
        onehots = {}     # b -> {(branch, ci): tile}
        z_sb = {}        # b -> {(branch, ci): tile}
        h1_pair = {}     # pair_idx -> [4 tiles [128, 392] bf16]

        def emit_front(b):
            """idx broadcast + one-hot + x load/cast + projection matmuls."""
            oh = {}
            for nm, br in (("x", 0), ("y", 1)):
                row = idx_dram[nm][b:b + 1, :]
                bcast = bcpool.tile([128, S], bf16, name=f"bc{nm}_{b}", tag=f"bc{nm}")
                nc.gpsimd.dma_start(
                    bcast[:], bass.AP(row.tensor, row.offset, [[0, 128], [1, S]])
                )
                for ci, (off, sz) in enumerate(HW_CHUNKS):
                    o = ohpool.tile([sz, S], bf16, name=f"oh{nm}{ci}_{b}",
                                    tag=f"oh{nm}{ci}")
                    iot = (iota_lo, iota_hi)[ci]
                    nc.vector.tensor_scalar(
                        o[:], bcast[0:sz, :], iot[:], None, op0=ALU.is_equal
                    )
                    oh[(br, ci)] = o
            onehots[b] = oh

            xs = xpool.tile([128, KC, S], f32, name=f"xs_{b}", tag="xs")
            for k in range(KC):
                nc.sync.dma_start(xs[:, k, :], x_t[b, k * 128:(k + 1) * 128, :])
            xbf = xbfpool.tile([128, KC, S], bf16, name=f"xbf_{b}", tag="xbf")
            nc.vector.tensor_copy(xbf[:], xs[:])

            # projection: z[branch][ci] [sz, 512] += x_chunk^T @ W1 chunk
            zt = {}
            for br in range(2):
                for ci, (off, sz) in enumerate(HW_CHUNKS):
                    zp = zps.tile([sz, HID], f32, name=f"zp{br}{ci}_{b}",
                                  tag=f"zp{br}{ci}")
                    for k in range(KC):
                        nc.tensor.matmul(
                            zp[:],
                            xbf[:, k, off:off + sz],
                            w1b[br * KC + k][:],
                            start=(k == 0),
                            stop=(k == KC - 1),
                        )
                    zs = zsb.tile([sz, HID], bf16, name=f"zs{br}{ci}_{b}",
                                  tag=f"zs{br}{ci}")
                    # alternate copy engine: ACT for lo chunks, DVE for hi
                    if ci == 0:
                        nc.scalar.copy(zs[:], zp[:])
                    else:
                        nc.vector.tensor_copy(zs[:], zp[:])
                    zt[(br, ci)] = zs
            z_sb[b] = zt

        def emit_gather(b):
            """one-hot gather matmuls + bias+relu into the pair tile."""
            pair, col = divmod(b, 2)
            if col == 0:
                h1_pair[pair] = [
                    h1pool.tile([128, 2 * S], bf16, name=f"h1p{pair}_{j}",
                                tag=f"h1p{j}")
                    for j in range(MJ)
                ]
            zt = z_sb.pop(b)
            oh = onehots.pop(b)
            for j in range(MJ):
                hp = hps.tile([128, S], f32, name=f"h1ps{j}_{b}", tag=f"hps{j}")
                n_mm = 0
                for br in range(2):
                    for ci, (off, sz) in enumerate(HW_CHUNKS):
                        nc.tensor.matmul(
                            hp[:],
                            zt[(br, ci)][:, j * 128:(j + 1) * 128],
                            oh[(br, ci)][:],
                            start=(n_mm == 0),
                            stop=(n_mm == 3),
                        )
                        n_mm += 1
                dst = h1_pair[pair][j][:, col * S:(col + 1) * S]
                if j % 2 == 0:
                    nc.scalar.activation(dst, hp[:], AF.Relu, bias=b1c[j][:])
                else:
                    nc.vector.tensor_scalar(
                        dst, hp[:], b1c[j][:], 0.0, op0=ALU.add, op1=ALU.max
                    )

        def emit_tail(pair):
            """GEMM2 + relu, GEMM3 + bias, DMA out for a finished pair."""
            h1t = h1_pair.pop(pair)
            h2t = []
            for j in range(MJ):
                hp = hps.tile([128, 2 * S], f32, name=f"h2ps{j}_p{pair}",
                              tag=f"hps{j}")
                for k in range(MJ):
                    nc.tensor.matmul(
                        hp[:],
                        w2b[k][:, j * 128:(j + 1) * 128],
                        h1t[k][:],
                        start=(k == 0),
                        stop=(k == MJ - 1),
                    )
                h2 = h2pool.tile([128, 2 * S], bf16, name=f"h2sb{j}_p{pair}",
                                 tag=f"h2sb{j}")
                if j % 2 == 0:
                    nc.scalar.activation(h2[:], hp[:], AF.Relu, bias=b2c[j][:])
                else:
                    nc.vector.tensor_scalar(
                        h2[:], hp[:], b2c[j][:], 0.0, op0=ALU.add, op1=ALU.max
                    )
                h2t.append(h2)
            pp = hps.tile([OUT, 2 * S], f32, name=f"predps_p{pair}", tag="hps0")
            for k in range(MJ):
                nc.tensor.matmul(
                    pp[:], w3b[k][:], h2t[k][:], start=(k == 0), stop=(k == MJ - 1)
                )
            po = opool.tile([OUT, 2 * S], f32, name=f"predsb_p{pair}", tag="predsb")
            nc.vector.tensor_scalar(po[:], pp[:], b3c[:], None, op0=ALU.add)
            nc.sync.dma_start(
                bass.AP(pred_t, pair * 2 * S * OUT, [[1, OUT], [OUT, 2 * S]]),
                po[:],
            )

        # ---------------- main loop: software-pipelined by one batch --------
        for b in range(BPC):
            emit_front(b)
            if b >= 1:
                emit_gather(b - 1)
                if (b - 1) % 2 == 1:
                    emit_tail((b - 1) // 2)
        emit_gather(BPC - 1)
        emit_tail((BPC - 1) // 2)

    nc.finalize()
    return nc


_NC = None


def _get_nc():
    global _NC
    if _NC is None:
        _install_ntff_hook()
        _NC = _build_nc()
    return _NC


def _make_in_maps(inputs):
    x = np.asarray(inputs["x"], dtype=np.float32).reshape(B, C, H * W_IMG)
    pxs = np.asarray(inputs["pxs"], dtype=np.int32).reshape(B, 2 * S)
    pys = np.asarray(inputs["pys"], dtype=np.int32).reshape(B, 2 * S)
    W1 = np.asarray(inputs["W1"], dtype=np.float32)
    W2 = np.asarray(inputs["W2"], dtype=np.float32)
    W3 = np.asarray(inputs["W3"], dtype=np.float32)
    b1 = np.asarray(inputs["b1"], dtype=np.float32)
    b2 = np.asarray(inputs["b2"], dtype=np.float32)
    b3 = np.asarray(inputs["b3"], dtype=np.float32)
    iota = np.arange(S, dtype=np.float32)
    in_maps = []
    for c in range(N_CORES):
        sl = slice(c * BPC, (c + 1) * BPC)
        in_maps.append({
            "x": np.ascontiguousarray(x[sl]),
            "pxs": np.ascontiguousarray(pxs[sl]),
            "pys": np.ascontiguousarray(pys[sl]),
            "W1": W1, "W2": W2, "W3": W3,
            "b1": b1, "b2": b2, "b3": b3,
            "iota196": iota,
        })
    return in_maps


def _run(inputs, trace=False):
    from concourse.bass_utils import run_bass_kernel_spmd

    nc = _get_nc()
    in_maps = _make_in_maps(inputs)
    res = run_bass_kernel_spmd(
        nc, in_maps, core_ids=list(range(N_CORES)), trace=trace
    )
    pred = np.concatenate(
        [res.results[c]["predxy"] for c in range(N_CORES)], axis=0
    )
    delta = np.concatenate(
        [res.results[c]["deltaxy"] for c in range(N_CORES)], axis=0
    )
    return (pred, delta), res


def kernel(**inputs):
    (pred, delta), _ = _run(inputs, trace=False)
    return pred, delta


# revision 3
# speedup vs baseline: 1.3644x; 1.3644x over previous
"""Trainium2 Bass kernel for nn_DenseRelativeLoc.

Data-parallel over batch: 32 images per NeuronCore x 8 cores.

Per image b (all activations kept transposed, feature-dim on partitions):
    z1a = x_flat[b]^T @ W1[:768]    [196hw, 512]   (natural layouts, no transposes)
    z1b = x_flat[b]^T @ W1[768:]    [196hw, 512]
    h1T[m, s] = relu( sum_hw z1a[hw, m]*onehot_x[hw, s]
                    + sum_hw z1b[hw, m]*onehot_y[hw, s] + b1[m] )
    (the gather of sampled points is folded into a one-hot matmul; this is
     exact because onehot selects the hw = idx[s] row)
    h2T = relu(W2^T @ h1T + b2)     [512, 196]
    predT = W3^T @ h2T + b3         [2, 196]
All matmuls in bf16 with fp32 PSUM accumulation.

One-hot construction (per pair of batches): idx rows are broadcast across
partitions with a K=1 outer-product matmul (ones[1,128]^T @ idx[1,392] ->
PSUM), then compared against a per-partition iota column with is_equal.

deltaxy = float(pxs - pys) + 13 computed on-device with DVE ops.
"""
import sys
import types

import numpy as np

B, C, H, W_IMG = 256, 768, 14, 14
S = 196          # sample count == H*W
HID = 512
OUT = 2
N_CORES = 8
BPC = B // N_CORES      # batches per core
KC = C // 128           # 6 contraction chunks for layer 1
MJ = HID // 128         # 4 HID chunks
HW_CHUNKS = [(0, 128), (128, 68)]   # hw partition chunks of 196
XG = 4                  # batches per x-load group
S2 = 2 * S              # pair width


def _install_ntff_hook():
    try:
        import antenv.axon_hooks  # noqa: F401
        return
    except ImportError:
        pass
    try:
        from trn_agent_boot.trn_boot import _ntff_profile_via_ctypes
        hook = _ntff_profile_via_ctypes("/opt/axon/libaxon_pjrt.so")
    except Exception:
        hook = None
    mod = types.ModuleType("antenv.axon_hooks")
    mod.get_axon_ntff_profile_hook = lambda: hook
    sys.modules["antenv.axon_hooks"] = mod


def _build_nc():
    from contextlib import ExitStack

    import concourse.bass as bass
    import concourse.bacc as bacc
    import concourse.mybir as mybir
    import concourse.tile as tile

    dt = mybir.dt
    f32, bf16, i32 = dt.float32, dt.bfloat16, dt.int32
    AF = mybir.ActivationFunctionType
    ALU = mybir.AluOpType

    nc = bacc.Bacc(None, target_bir_lowering=False)

    # x comes host-permuted as [KC, 128, BPC*S] so DMA segments are large
    x_t = nc.dram_tensor("x", [KC, 128, BPC * S], f32, kind="ExternalInput")
    pxs_t = nc.dram_tensor("pxs", [BPC, S2], i32, kind="ExternalInput")
    pys_t = nc.dram_tensor("pys", [BPC, S2], i32, kind="ExternalInput")
    w1_t = nc.dram_tensor("W1", [2 * C, HID], f32, kind="ExternalInput")
    w2_t = nc.dram_tensor("W2", [HID, HID], f32, kind="ExternalInput")
    w3_t = nc.dram_tensor("W3", [HID, OUT], f32, kind="ExternalInput")
    b1_t = nc.dram_tensor("b1", [HID], f32, kind="ExternalInput")
    b2_t = nc.dram_tensor("b2", [HID], f32, kind="ExternalInput")
    b3_t = nc.dram_tensor("b3", [OUT], f32, kind="ExternalInput")
    iota_t = nc.dram_tensor("iota196", [S], f32, kind="ExternalInput")
    # predT: [2, BPC*S] (component-major); host transposes after gather
    pred_t = nc.dram_tensor("predT", [OUT, BPC * S], f32, kind="ExternalOutput")
    delta_t = nc.dram_tensor("deltaxy", [BPC * S, OUT], f32, kind="ExternalOutput")

    with ExitStack() as ctx:
        tc = ctx.enter_context(tile.TileContext(nc))
        wpool = ctx.enter_context(tc.tile_pool(name="w", bufs=1))
        wstage = ctx.enter_context(tc.tile_pool(name="wstage", bufs=2))
        xpool = ctx.enter_context(tc.tile_pool(name="x", bufs=2))
        xbfpool = ctx.enter_context(tc.tile_pool(name="xbf", bufs=2))
        zsb = ctx.enter_context(tc.tile_pool(name="zsb", bufs=2))
        ohpool = ctx.enter_context(tc.tile_pool(name="oh", bufs=2))
        h1pool = ctx.enter_context(tc.tile_pool(name="h1", bufs=2))
        h2pool = ctx.enter_context(tc.tile_pool(name="h2", bufs=2))
        opool = ctx.enter_context(tc.tile_pool(name="op", bufs=1))
        idxpool = ctx.enter_context(tc.tile_pool(name="idx", bufs=1))
        zps = ctx.enter_context(tc.tile_pool(name="zps", bufs=1, space="PSUM"))
        hps = ctx.enter_context(tc.tile_pool(name="hps", bufs=1, space="PSUM"))
        bcps = ctx.enter_context(tc.tile_pool(name="bcps", bufs=2, space="PSUM"))
        dram = ctx.enter_context(tc.tile_pool(name="dram", bufs=1, space="DRAM"))

        # ---------------- weights: load fp32, cast to bf16 on-chip ----------
        w1b = []
        for k in range(2 * KC):
            st = wstage.tile([128, HID], f32, name=f"w1st{k}", tag="wst")
            nc.sync.dma_start(st[:], w1_t[k * 128:(k + 1) * 128, :])
            wb = wpool.tile([128, HID], bf16, name=f"w1b{k}", tag=f"w1b{k}")
            nc.vector.tensor_copy(wb[:], st[:])
            w1b.append(wb)
        w2b = []
        for k in range(MJ):
            st = wstage.tile([128, HID], f32, name=f"w2st{k}", tag="wst")
            nc.sync.dma_start(st[:], w2_t[k * 128:(k + 1) * 128, :])
            wb = wpool.tile([128, HID], bf16, name=f"w2b{k}", tag=f"w2b{k}")
            nc.vector.tensor_copy(wb[:], st[:])
            w2b.append(wb)
        w3b = []
        for k in range(MJ):
            st = wstage.tile([128, OUT], f32, name=f"w3st{k}", tag="w3st")
            nc.sync.dma_start(st[:], w3_t[k * 128:(k + 1) * 128, :])
            wb = wpool.tile([128, OUT], bf16, name=f"w3b{k}", tag=f"w3b{k}")
            nc.vector.tensor_copy(wb[:], st[:])
            w3b.append(wb)
        b1c, b2c = [], []
        for j in range(MJ):
            t1 = wpool.tile([128, 1], f32, name=f"b1c{j}", tag=f"b1c{j}")
            nc.sync.dma_start(t1[:], b1_t[j * 128:(j + 1) * 128])
            b1c.append(t1)
            t2 = wpool.tile([128, 1], f32, name=f"b2c{j}", tag=f"b2c{j}")
            nc.sync.dma_start(t2[:], b2_t[j * 128:(j + 1) * 128])
            b2c.append(t2)
        b3c = wpool.tile([OUT, 1], f32, name="b3c", tag="b3c")
        nc.sync.dma_start(b3c[:], b3_t[:])
        iota_lo = wpool.tile([128, 1], f32, name="iota_lo", tag="iota_lo")
        nc.sync.dma_start(iota_lo[:], iota_t[0:128])
        iota_hi = wpool.tile([68, 1], f32, name="iota_hi", tag="iota_hi")
        nc.sync.dma_start(iota_hi[:], iota_t[128:S])
        ones_row = wpool.tile([1, 128], bf16, name="ones_row", tag="ones_row")
        nc.vector.memset(ones_row[:], 1.0)

        # ---------------- per-core index prep + deltaxy ---------------------
        pxs_sb = idxpool.tile([BPC, S2], i32, name="pxs_sb", tag="pxs_sb")
        nc.sync.dma_start(pxs_sb[:], pxs_t[:, :])
        pys_sb = idxpool.tile([BPC, S2], i32, name="pys_sb", tag="pys_sb")
        nc.sync.dma_start(pys_sb[:], pys_t[:, :])

        dsub = idxpool.tile([BPC, S2], i32, name="dsub", tag="dsub")
        nc.vector.tensor_tensor(dsub[:], pxs_sb[:], pys_sb[:], ALU.subtract)
        ddel = idxpool.tile([BPC, S2], f32, name="ddel", tag="ddel")
        nc.vector.tensor_scalar(ddel[:], dsub[:], float(H - 1), None, op0=ALU.add)
        nc.sync.dma_start(bass.AP(delta_t, 0, [[S2, BPC], [1, S2]]), ddel[:])

        # idx = px_row*14 + px_col as bf16 (exact: values < 256); round-trip
        # through DRAM to flatten [BPC, S] (batch-on-partition) to one row.
        idx_rows = {}
        for nm, src in (("x", pxs_sb), ("y", pys_sb)):
            tmp = idxpool.tile([BPC, S], i32, name=f"itmp{nm}", tag=f"itmp{nm}")
            nc.vector.tensor_scalar(
                tmp[:], src[:, 0:S2:2], float(W_IMG), None, op0=ALU.mult
            )
            ib = idxpool.tile([BPC, S], bf16, name=f"ibf{nm}", tag=f"ibf{nm}")
            nc.vector.tensor_tensor(ib[:], tmp[:], src[:, 1:S2:2], ALU.add)
            d = dram.tile([BPC, S], bf16, name=f"idxd{nm}", tag=f"idxd{nm}")
            nc.sync.dma_start(d[:, :], ib[:])
            row = idxpool.tile([1, BPC * S], bf16, name=f"irow{nm}", tag=f"irow{nm}")
            nc.sync.dma_start(row[:], bass.AP(d.tensor, d[:, :].offset,
                                              [[BPC * S, 1], [1, BPC * S]]))
            idx_rows[nm] = row

        pred_all = opool.tile([OUT, BPC * S], f32, name="pred_all", tag="pred_all")

        # ---------------- per-batch emission helpers ------------------------
        onehots = {}     # pair -> {(branch, ci): tile [sz, S2]}
        z_sb = {}        # b -> {(branch, ci): tile}
        h1_pair = {}     # pair -> [4 tiles [128, S2] bf16]
        xbf_g = {}       # group -> tile [128, KC, XG*S] bf16

        def emit_xgroup(g):
            xs = xpool.tile([128, KC, XG * S], f32, name=f"xs_{g}", tag="xs")
            nc.sync.dma_start(
                xs[:],
                bass.AP(x_t, g * XG * S,
                        [[BPC * S, 128], [128 * BPC * S, KC], [1, XG * S]]),
            )
            xb = xbfpool.tile([128, KC, XG * S], bf16, name=f"xbf_{g}", tag="xbf")
            nc.vector.tensor_copy(xb[:], xs[:])
            xbf_g[g] = xb

        def emit_onehots(pair):
            """Outer-product broadcast + is_equal one-hots for a batch pair."""
            oh = {}
            for nm, br in (("x", 0), ("y", 1)):
                bc = bcps.tile([128, S2], f32, name=f"bc{nm}_{pair}", tag="bcps")
                nc.tensor.matmul(
                    bc[:], ones_row[:],
                    idx_rows[nm][:, pair * S2:(pair + 1) * S2],
                    start=True, stop=True,
                )
                for ci, (off, sz) in enumerate(HW_CHUNKS):
                    o = ohpool.tile([sz, S2], bf16, name=f"oh{nm}{ci}_{pair}",
                                    tag=f"oh{nm}{ci}")
                    iot = (iota_lo, iota_hi)[ci]
                    nc.vector.tensor_scalar(
                        o[:], bc[0:sz, :], iot[:], None, op0=ALU.is_equal
                    )
                    oh[(br, ci)] = o
            onehots[pair] = oh

        def emit_proj(b, ci):
            """Projection matmuls for one hw chunk of batch b + PSUM->SBUF."""
            off, sz = HW_CHUNKS[ci]
            g, gi = divmod(b, XG)
            xb = xbf_g[g]
            zt = z_sb.setdefault(b, {})
            for br in range(2):
                zp = zps.tile([sz, HID], f32, name=f"zp{br}{ci}_{b}",
                              tag=f"zp{br}")
                for k in range(KC):
                    nc.tensor.matmul(
                        zp[:],
                        xb[:, k, gi * S + off:gi * S + off + sz],
                        w1b[br * KC + k][:],
                        start=(k == 0),
                        stop=(k == KC - 1),
                    )
                zs = zsb.tile([sz, HID], bf16, name=f"zs{br}{ci}_{b}",
                              tag=f"zs{br}{ci}")
                if br == 0:
                    nc.scalar.copy(zs[:], zp[:])
                else:
                    nc.vector.tensor_copy(zs[:], zp[:])
                zt[(br, ci)] = zs

        def emit_gather(b):
            """one-hot gather matmuls + bias+relu into the pair tile."""
            pair, col = divmod(b, 2)
            if col == 0:
                h1_pair[pair] = [
                    h1pool.tile([128, S2], bf16, name=f"h1p{pair}_{j}",
                                tag=f"h1p{j}")
                    for j in range(MJ)
                ]
            zt = z_sb.pop(b)
            oh = onehots[pair]
            if col == 1:
                onehots.pop(pair)
            for j in range(MJ):
                hp = hps.tile([128, S], f32, name=f"h1ps{j}_{b}", tag=f"hps{j}")
                n_mm = 0
                for br in range(2):
                    for ci, (off, sz) in enumerate(HW_CHUNKS):
                        nc.tensor.matmul(
                            hp[:],
                            zt[(br, ci)][:, j * 128:(j + 1) * 128],
                            oh[(br, ci)][:, col * S:(col + 1) * S],
                            start=(n_mm == 0),
                            stop=(n_mm == 3),
                        )
                        n_mm += 1
                dst = h1_pair[pair][j][:, col * S:(col + 1) * S]
                if j % 2 == 0:
                    nc.scalar.activation(dst, hp[:], AF.Relu, bias=b1c[j][:])
                else:
                    nc.vector.tensor_scalar(
                        dst, hp[:], b1c[j][:], 0.0, op0=ALU.add, op1=ALU.max
                    )

        def emit_tail(pair):
            """GEMM2 + relu, GEMM3 + bias into pred_all, for a finished pair."""
            h1t = h1_pair.pop(pair)
            h2t = []
            for j in range(MJ):
                hp = hps.tile([128, S2], f32, name=f"h2ps{j}_p{pair}",
                              tag=f"hps{j}")
                for k in range(MJ):
                    nc.tensor.matmul(
                        hp[:],
                        w2b[k][:, j * 128:(j + 1) * 128],
                        h1t[k][:],
                        start=(k == 0),
                        stop=(k == MJ - 1),
                    )
                h2 = h2pool.tile([128, S2], bf16, name=f"h2sb{j}_p{pair}",
                                 tag=f"h2sb{j}")
                if j % 2 == 0:
                    nc.scalar.activation(h2[:], hp[:], AF.Relu, bias=b2c[j][:])
                else:
                    nc.vector.tensor_scalar(
                        h2[:], hp[:], b2c[j][:], 0.0, op0=ALU.add, op1=ALU.max
                    )
                h2t.append(h2)
            pp = hps.tile([OUT, S2], f32, name=f"predps_p{pair}", tag="hps0")
            for k in range(MJ):
                nc.tensor.matmul(
                    pp[:], w3b[k][:], h2t[k][:], start=(k == 0), stop=(k == MJ - 1)
                )
            nc.vector.tensor_scalar(
                pred_all[:, pair * S2:(pair + 1) * S2], pp[:], b3c[:], None,
                op0=ALU.add,
            )

        # ---------------- main loop: software-pipelined by one batch --------
        # PE order per b: [pair front: onehot bcast MMs] proj_lo(b),
        # gather(b-1), proj_hi(b), [pair tail: gemm2+gemm3]
        emit_xgroup(0)
        for b in range(BPC):
            if b % XG == 0 and b // XG + 1 < (BPC + XG - 1) // XG:
                emit_xgroup(b // XG + 1)
            if b % 2 == 0:
                emit_onehots(b // 2)
            emit_proj(b, 0)
            if b >= 1:
                emit_gather(b - 1)
            emit_proj(b, 1)
            if b >= 1 and (b - 1) % 2 == 1:
                emit_tail((b - 1) // 2)
        emit_gather(BPC - 1)
        emit_tail((BPC - 1) // 2)

        nc.sync.dma_start(pred_t[:, :], pred_all[:])

    nc.finalize()
    return nc


_NC = None


def _get_nc():
    global _NC
    if _NC is None:
        _install_ntff_hook()
        _NC = _build_nc()
    return _NC


def _make_in_maps(inputs):
    x = np.asarray(inputs["x"], dtype=np.float32).reshape(B, C, H * W_IMG)
    pxs = np.asarray(inputs["pxs"], dtype=np.int32).reshape(B, S2)
    pys = np.asarray(inputs["pys"], dtype=np.int32).reshape(B, S2)
    W1 = np.asarray(inputs["W1"], dtype=np.float32)
    W2 = np.asarray(inputs["W2"], dtype=np.float32)
    W3 = np.asarray(inputs["W3"], dtype=np.float32)
    b1 = np.asarray(inputs["b1"], dtype=np.float32)
    b2 = np.asarray(inputs["b2"], dtype=np.float32)
    b3 = np.asarray(inputs["b3"], dtype=np.float32)
    iota = np.arange(S, dtype=np.float32)
    in_maps = []
    for c in range(N_CORES):
        sl = slice(c * BPC, (c + 1) * BPC)
        # [BPC, C, S] -> [KC, 128, BPC*S] so per-partition DMA segments are
        # BPC*S contiguous floats per channel
        xc = np.ascontiguousarray(
            x[sl].reshape(BPC, KC, 128, S).transpose(1, 2, 0, 3)
        ).reshape(KC, 128, BPC * S)
        in_maps.append({
            "x": xc,
            "pxs": np.ascontiguousarray(pxs[sl]),
            "pys": np.ascontiguousarray(pys[sl]),
            "W1": W1, "W2": W2, "W3": W3,
            "b1": b1, "b2": b2, "b3": b3,
            "iota196": iota,
        })
    return in_maps


def _run(inputs, trace=False):
    from concourse.bass_utils import run_bass_kernel_spmd

    nc = _get_nc()
    in_maps = _make_in_maps(inputs)
    res = run_bass_kernel_spmd(
        nc, in_maps, core_ids=list(range(N_CORES)), trace=trace
    )
    pred = np.concatenate(
        [np.ascontiguousarray(res.results[c]["predT"].T) for c in range(N_CORES)],
        axis=0,
    )
    delta = np.concatenate(
        [res.results[c]["deltaxy"] for c in range(N_CORES)], axis=0
    )
    return (pred, delta), res


def kernel(**inputs):
    (pred, delta), _ = _run(inputs, trace=False)
    return pred, delta


# revision 4
# speedup vs baseline: 1.3914x; 1.0198x over previous
"""Trainium2 Bass kernel for nn_DenseRelativeLoc.

Data-parallel over batch: 32 images per NeuronCore x 8 cores.

Per image b (all activations kept transposed, feature-dim on partitions):
    z1a = x_flat[b]^T @ W1[:768]    [196hw, 512]   (natural layouts, no transposes)
    z1b = x_flat[b]^T @ W1[768:]    [196hw, 512]
    h1T[m, s] = relu( sum_hw z1a[hw, m]*onehot_x[hw, s]
                    + sum_hw z1b[hw, m]*onehot_y[hw, s] + b1[m] )
    (the gather of sampled points is folded into a one-hot matmul; this is
     exact because onehot selects the hw = idx[s] row)
    h2T = relu(W2^T @ h1T + b2)     [512, 196]
    predT = W3^T @ h2T + b3         [2, 196]
All matmuls in bf16 with fp32 PSUM accumulation.

One-hot construction (per pair of batches): idx rows are broadcast across
partitions with a K=1 outer-product matmul (ones[1,128]^T @ idx[1,392] ->
PSUM), then compared against a per-partition iota column with is_equal.

deltaxy = float(pxs - pys) + 13 computed on-device with DVE ops.
"""
import sys
import types

import numpy as np

B, C, H, W_IMG = 256, 768, 14, 14
S = 196          # sample count == H*W
HID = 512
OUT = 2
N_CORES = 8
BPC = B // N_CORES      # batches per core
KC = C // 128           # 6 contraction chunks for layer 1
MJ = HID // 128         # 4 HID chunks
HW_CHUNKS = [(0, 128), (128, 68)]   # hw partition chunks of 196
XG = 4                  # batches per steady-state x-load group
GROUPS = [(0, 1), (1, 3)] + [(4 + 4 * i, 4) for i in range((BPC - 4) // 4)]
B2G = {}
for _gi, (_st, _ln) in enumerate(GROUPS):
    for _b in range(_st, _st + _ln):
        B2G[_b] = (_gi, _b - _st)
S2 = 2 * S              # pair width


def _install_ntff_hook():
    try:
        import antenv.axon_hooks  # noqa: F401
        return
    except ImportError:
        pass
    try:
        from trn_agent_boot.trn_boot import _ntff_profile_via_ctypes
        hook = _ntff_profile_via_ctypes("/opt/axon/libaxon_pjrt.so")
    except Exception:
        hook = None
    mod = types.ModuleType("antenv.axon_hooks")
    mod.get_axon_ntff_profile_hook = lambda: hook
    sys.modules["antenv.axon_hooks"] = mod


def _build_nc():
    from contextlib import ExitStack

    import concourse.bass as bass
    import concourse.bacc as bacc
    import concourse.mybir as mybir
    import concourse.tile as tile

    dt = mybir.dt
    f32, bf16, i32 = dt.float32, dt.bfloat16, dt.int32
    AF = mybir.ActivationFunctionType
    ALU = mybir.AluOpType

    nc = bacc.Bacc(None, target_bir_lowering=False)

    # x comes host-permuted as [KC, 128, BPC*S] so DMA segments are large
    x_t = nc.dram_tensor("x", [KC, 128, BPC * S], f32, kind="ExternalInput")
    pxs_t = nc.dram_tensor("pxs", [BPC, S2], i32, kind="ExternalInput")
    pys_t = nc.dram_tensor("pys", [BPC, S2], i32, kind="ExternalInput")
    w1_t = nc.dram_tensor("W1", [2 * C, HID], f32, kind="ExternalInput")
    w2_t = nc.dram_tensor("W2", [HID, HID], f32, kind="ExternalInput")
    w3_t = nc.dram_tensor("W3", [HID, OUT], f32, kind="ExternalInput")
    b1_t = nc.dram_tensor("b1", [HID], f32, kind="ExternalInput")
    b2_t = nc.dram_tensor("b2", [HID], f32, kind="ExternalInput")
    b3_t = nc.dram_tensor("b3", [OUT], f32, kind="ExternalInput")
    iota_t = nc.dram_tensor("iota196", [S], f32, kind="ExternalInput")
    # predT: [2, BPC*S] (component-major); host transposes after gather
    pred_t = nc.dram_tensor("predT", [OUT, BPC * S], f32, kind="ExternalOutput")
    delta_t = nc.dram_tensor("deltaxy", [BPC * S, OUT], f32, kind="ExternalOutput")

    with ExitStack() as ctx:
        tc = ctx.enter_context(tile.TileContext(nc))
        wpool = ctx.enter_context(tc.tile_pool(name="w", bufs=1))
        wstage = ctx.enter_context(tc.tile_pool(name="wstage", bufs=2))
        xpool = ctx.enter_context(tc.tile_pool(name="x", bufs=2))
        xbfpool = ctx.enter_context(tc.tile_pool(name="xbf", bufs=2))
        zsb = ctx.enter_context(tc.tile_pool(name="zsb", bufs=2))
        ohpool = ctx.enter_context(tc.tile_pool(name="oh", bufs=2))
        h1pool = ctx.enter_context(tc.tile_pool(name="h1", bufs=2))
        h2pool = ctx.enter_context(tc.tile_pool(name="h2", bufs=2))
        opool = ctx.enter_context(tc.tile_pool(name="op", bufs=1))
        idxpool = ctx.enter_context(tc.tile_pool(name="idx", bufs=1))
        zps = ctx.enter_context(tc.tile_pool(name="zps", bufs=1, space="PSUM"))
        hps = ctx.enter_context(tc.tile_pool(name="hps", bufs=1, space="PSUM"))
        bcps = ctx.enter_context(tc.tile_pool(name="bcps", bufs=2, space="PSUM"))
        dram = ctx.enter_context(tc.tile_pool(name="dram", bufs=1, space="DRAM"))

        # ---------------- early: batch-0 x slab so PE can start fast --------
        xpf = xpool.tile([128, KC, 1 * S], f32, name="xs_0", tag="xs",
                         padded_shape=[128, KC, XG * S])
        nc.sync.dma_start(
            xpf[:],
            bass.AP(x_t, 0, [[BPC * S, 128], [128 * BPC * S, KC], [1, S]]),
        )
        # ---------------- weights: load fp32, cast to bf16 on-chip ----------
        w1b = []
        for k in range(2 * KC):
            st = wstage.tile([128, HID], f32, name=f"w1st{k}", tag="wst")
            nc.sync.dma_start(st[:], w1_t[k * 128:(k + 1) * 128, :])
            wb = wpool.tile([128, HID], bf16, name=f"w1b{k}", tag=f"w1b{k}")
            nc.vector.tensor_copy(wb[:], st[:])
            w1b.append(wb)
        w2b = []
        for k in range(MJ):
            st = wstage.tile([128, HID], f32, name=f"w2st{k}", tag="wst")
            nc.sync.dma_start(st[:], w2_t[k * 128:(k + 1) * 128, :])
            wb = wpool.tile([128, HID], bf16, name=f"w2b{k}", tag=f"w2b{k}")
            nc.vector.tensor_copy(wb[:], st[:])
            w2b.append(wb)
        w3b = []
        for k in range(MJ):
            st = wstage.tile([128, OUT], f32, name=f"w3st{k}", tag="w3st")
            nc.sync.dma_start(st[:], w3_t[k * 128:(k + 1) * 128, :])
            wb = wpool.tile([128, OUT], bf16, name=f"w3b{k}", tag=f"w3b{k}")
            nc.vector.tensor_copy(wb[:], st[:])
            w3b.append(wb)
        b1c, b2c = [], []
        for j in range(MJ):
            t1 = wpool.tile([128, 1], f32, name=f"b1c{j}", tag=f"b1c{j}")
            nc.sync.dma_start(t1[:], b1_t[j * 128:(j + 1) * 128])
            b1c.append(t1)
            t2 = wpool.tile([128, 1], f32, name=f"b2c{j}", tag=f"b2c{j}")
            nc.sync.dma_start(t2[:], b2_t[j * 128:(j + 1) * 128])
            b2c.append(t2)
        b3c = wpool.tile([OUT, 1], f32, name="b3c", tag="b3c")
        nc.sync.dma_start(b3c[:], b3_t[:])
        iota_lo = wpool.tile([128, 1], f32, name="iota_lo", tag="iota_lo")
        nc.sync.dma_start(iota_lo[:], iota_t[0:128])
        iota_hi = wpool.tile([68, 1], f32, name="iota_hi", tag="iota_hi")
        nc.sync.dma_start(iota_hi[:], iota_t[128:S])
        ones_row = wpool.tile([1, 128], bf16, name="ones_row", tag="ones_row")
        nc.vector.memset(ones_row[:], 1.0)

        # ---------------- per-core index prep + deltaxy ---------------------
        pxs_sb = idxpool.tile([BPC, S2], i32, name="pxs_sb", tag="pxs_sb")
        nc.sync.dma_start(pxs_sb[:], pxs_t[:, :])
        pys_sb = idxpool.tile([BPC, S2], i32, name="pys_sb", tag="pys_sb")
        nc.sync.dma_start(pys_sb[:], pys_t[:, :])

        dsub = idxpool.tile([BPC, S2], i32, name="dsub", tag="dsub")
        nc.vector.tensor_tensor(dsub[:], pxs_sb[:], pys_sb[:], ALU.subtract)
        ddel = idxpool.tile([BPC, S2], f32, name="ddel", tag="ddel")
        nc.vector.tensor_scalar(ddel[:], dsub[:], float(H - 1), None, op0=ALU.add)
        nc.sync.dma_start(bass.AP(delta_t, 0, [[S2, BPC], [1, S2]]), ddel[:])

        # idx = px_row*14 + px_col as bf16 (exact: values < 256); round-trip
        # through DRAM to flatten [BPC, S] (batch-on-partition) to one row.
        idx_rows = {}
        for nm, src in (("x", pxs_sb), ("y", pys_sb)):
            tmp = idxpool.tile([BPC, S], i32, name=f"itmp{nm}", tag=f"itmp{nm}")
            nc.vector.tensor_scalar(
                tmp[:], src[:, 0:S2:2], float(W_IMG), None, op0=ALU.mult
            )
            ib = idxpool.tile([BPC, S], bf16, name=f"ibf{nm}", tag=f"ibf{nm}")
            nc.vector.tensor_tensor(ib[:], tmp[:], src[:, 1:S2:2], ALU.add)
            d = dram.tile([BPC, S], bf16, name=f"idxd{nm}", tag=f"idxd{nm}")
            nc.sync.dma_start(d[:, :], ib[:])
            row = idxpool.tile([1, BPC * S], bf16, name=f"irow{nm}", tag=f"irow{nm}")
            nc.sync.dma_start(row[:], bass.AP(d.tensor, d[:, :].offset,
                                              [[BPC * S, 1], [1, BPC * S]]))
            idx_rows[nm] = row

        pred_all = opool.tile([OUT, BPC * S], f32, name="pred_all", tag="pred_all")

        # ---------------- per-batch emission helpers ------------------------
        onehots = {}     # pair -> {(branch, ci): tile [sz, S2]}
        z_sb = {}        # b -> {(branch, ci): tile}
        h1_pair = {}     # pair -> [4 tiles [128, S2] bf16]
        xbf_g = {}       # group -> tile [128, KC, XG*S] bf16

        def emit_xgroup(g):
            st, ln = GROUPS[g]
            if g == 0:
                xs = xpf
            else:
                xs = xpool.tile([128, KC, ln * S], f32, name=f"xs_{g}", tag="xs",
                                padded_shape=[128, KC, XG * S])
                nc.sync.dma_start(
                    xs[:],
                    bass.AP(x_t, st * S,
                            [[BPC * S, 128], [128 * BPC * S, KC], [1, ln * S]]),
                )
            xb = xbfpool.tile([128, KC, ln * S], bf16, name=f"xbf_{g}", tag="xbf",
                              padded_shape=[128, KC, XG * S])
            nc.vector.tensor_copy(xb[:], xs[:])
            xbf_g[g] = xb

        def emit_onehots(pair):
            """Outer-product broadcast + is_equal one-hots for a batch pair."""
            oh = {}
            for nm, br in (("x", 0), ("y", 1)):
                bc = bcps.tile([128, S2], f32, name=f"bc{nm}_{pair}", tag="bcps")
                nc.tensor.matmul(
                    bc[:], ones_row[:],
                    idx_rows[nm][:, pair * S2:(pair + 1) * S2],
                    start=True, stop=True,
                )
                for ci, (off, sz) in enumerate(HW_CHUNKS):
                    o = ohpool.tile([sz, S2], bf16, name=f"oh{nm}{ci}_{pair}",
                                    tag=f"oh{nm}{ci}")
                    iot = (iota_lo, iota_hi)[ci]
                    nc.vector.tensor_scalar(
                        o[:], bc[0:sz, :], iot[:], None, op0=ALU.is_equal
                    )
                    oh[(br, ci)] = o
            onehots[pair] = oh

        def emit_proj(b, ci):
            """Projection matmuls for one hw chunk of batch b + PSUM->SBUF."""
            off, sz = HW_CHUNKS[ci]
            g, gi = B2G[b]
            xb = xbf_g[g]
            zt = z_sb.setdefault(b, {})
            for br in range(2):
                zp = zps.tile([sz, HID], f32, name=f"zp{br}{ci}_{b}",
                              tag=f"zp{br}")
                for k in range(KC):
                    nc.tensor.matmul(
                        zp[:],
                        xb[:, k, gi * S + off:gi * S + off + sz],
                        w1b[br * KC + k][:],
                        start=(k == 0),
                        stop=(k == KC - 1),
                    )
                zs = zsb.tile([sz, HID], bf16, name=f"zs{br}{ci}_{b}",
                              tag=f"zs{br}{ci}")
                if br == 0:
                    nc.scalar.copy(zs[:], zp[:])
                else:
                    nc.vector.tensor_copy(zs[:], zp[:])
                zt[(br, ci)] = zs

        def emit_gather(b):
            """one-hot gather matmuls + bias+relu into the pair tile."""
            pair, col = divmod(b, 2)
            if col == 0:
                h1_pair[pair] = [
                    h1pool.tile([128, S2], bf16, name=f"h1p{pair}_{j}",
                                tag=f"h1p{j}")
                    for j in range(MJ)
                ]
            zt = z_sb.pop(b)
            oh = onehots[pair]
            if col == 1:
                onehots.pop(pair)
            for j in range(MJ):
                hp = hps.tile([128, S], f32, name=f"h1ps{j}_{b}", tag=f"hps{j}")
                n_mm = 0
                for br in range(2):
                    for ci, (off, sz) in enumerate(HW_CHUNKS):
                        nc.tensor.matmul(
                            hp[:],
                            zt[(br, ci)][:, j * 128:(j + 1) * 128],
                            oh[(br, ci)][:, col * S:(col + 1) * S],
                            start=(n_mm == 0),
                            stop=(n_mm == 3),
                        )
                        n_mm += 1
                dst = h1_pair[pair][j][:, col * S:(col + 1) * S]
                if j % 2 == 0:
                    nc.scalar.activation(dst, hp[:], AF.Relu, bias=b1c[j][:])
                else:
                    nc.vector.tensor_scalar(
                        dst, hp[:], b1c[j][:], 0.0, op0=ALU.add, op1=ALU.max
                    )

        def emit_tail(pair):
            """GEMM2 + relu, GEMM3 + bias into pred_all, for a finished pair."""
            h1t = h1_pair.pop(pair)
            h2t = []
            for j in range(MJ):
                hp = hps.tile([128, S2], f32, name=f"h2ps{j}_p{pair}",
                              tag=f"hps{j}")
                for k in range(MJ):
                    nc.tensor.matmul(
                        hp[:],
                        w2b[k][:, j * 128:(j + 1) * 128],
                        h1t[k][:],
                        start=(k == 0),
                        stop=(k == MJ - 1),
                    )
                h2 = h2pool.tile([128, S2], bf16, name=f"h2sb{j}_p{pair}",
                                 tag=f"h2sb{j}")
                if j % 2 == 0:
                    nc.scalar.activation(h2[:], hp[:], AF.Relu, bias=b2c[j][:])
                else:
                    nc.vector.tensor_scalar(
                        h2[:], hp[:], b2c[j][:], 0.0, op0=ALU.add, op1=ALU.max
                    )
                h2t.append(h2)
            pp = hps.tile([OUT, S2], f32, name=f"predps_p{pair}", tag="hps0")
            for k in range(MJ):
                nc.tensor.matmul(
                    pp[:], w3b[k][:], h2t[k][:], start=(k == 0), stop=(k == MJ - 1)
                )
            nc.vector.tensor_scalar(
                pred_all[:, pair * S2:(pair + 1) * S2], pp[:], b3c[:], None,
                op0=ALU.add,
            )
            if pair % 4 == 3:
                q = pair // 4
                nc.sync.dma_start(
                    pred_t[:, q * 4 * S2:(q + 1) * 4 * S2],
                    pred_all[:, q * 4 * S2:(q + 1) * 4 * S2],
                )

        # ---------------- main loop: software-pipelined by one batch --------
        # PE order per b: [pair front: onehot bcast MMs] proj_lo(b),
        # gather(b-1), proj_hi(b), [pair tail: gemm2+gemm3]
        emit_xgroup(0)
        for b in range(BPC):
            g, gi = B2G[b]
            if gi == 0 and g + 1 < len(GROUPS):
                emit_xgroup(g + 1)
            if b % 2 == 0:
                emit_onehots(b // 2)
            emit_proj(b, 0)
            if b >= 1:
                emit_gather(b - 1)
            emit_proj(b, 1)
            if b >= 1 and (b - 1) % 2 == 1:
                emit_tail((b - 1) // 2)
        emit_gather(BPC - 1)
        emit_tail((BPC - 1) // 2)

    nc.finalize()
    return nc


_NC = None


def _get_nc():
    global _NC
    if _NC is None:
        _install_ntff_hook()
        _NC = _build_nc()
    return _NC


def _make_in_maps(inputs):
    x = np.asarray(inputs["x"], dtype=np.float32).reshape(B, C, H * W_IMG)
    pxs = np.asarray(inputs["pxs"], dtype=np.int32).reshape(B, S2)
    pys = np.asarray(inputs["pys"], dtype=np.int32).reshape(B, S2)
    W1 = np.asarray(inputs["W1"], dtype=np.float32)
    W2 = np.asarray(inputs["W2"], dtype=np.float32)
    W3 = np.asarray(inputs["W3"], dtype=np.float32)
    b1 = np.asarray(inputs["b1"], dtype=np.float32)
    b2 = np.asarray(inputs["b2"], dtype=np.float32)
    b3 = np.asarray(inputs["b3"], dtype=np.float32)
    iota = np.arange(S, dtype=np.float32)
    in_maps = []
    for c in range(N_CORES):
        sl = slice(c * BPC, (c + 1) * BPC)
        # [BPC, C, S] -> [KC, 128, BPC*S] so per-partition DMA segments are
        # BPC*S contiguous floats per channel
        xc = np.ascontiguousarray(
            x[sl].reshape(BPC, KC, 128, S).transpose(1, 2, 0, 3)
        ).reshape(KC, 128, BPC * S)
        in_maps.append({
            "x": xc,
            "pxs": np.ascontiguousarray(pxs[sl]),
            "pys": np.ascontiguousarray(pys[sl]),
            "W1": W1, "W2": W2, "W3": W3,
            "b1": b1, "b2": b2, "b3": b3,
            "iota196": iota,
        })
    return in_maps


def _run(inputs, trace=False):
    from concourse.bass_utils import run_bass_kernel_spmd

    nc = _get_nc()
    in_maps = _make_in_maps(inputs)
    res = run_bass_kernel_spmd(
        nc, in_maps, core_ids=list(range(N_CORES)), trace=trace
    )
    pred = np.concatenate(
        [np.ascontiguousarray(res.results[c]["predT"].T) for c in range(N_CORES)],
        axis=0,
    )
    delta = np.concatenate(
        [res.results[c]["deltaxy"] for c in range(N_CORES)], axis=0
    )
    return (pred, delta), res


def kernel(**inputs):
    (pred, delta), _ = _run(inputs, trace=False)
    return pred, delta


# revision 5
# speedup vs baseline: 1.3997x; 1.0059x over previous
"""Trainium2 Bass kernel for nn_DenseRelativeLoc.

Data-parallel over batch: 32 images per NeuronCore x 8 cores.

Per image b (all activations kept transposed, feature-dim on partitions):
    z1a = x_flat[b]^T @ W1[:768]    [196hw, 512]   (natural layouts, no transposes)
    z1b = x_flat[b]^T @ W1[768:]    [196hw, 512]
    h1T[m, s] = relu( sum_hw z1a[hw, m]*onehot_x[hw, s]
                    + sum_hw z1b[hw, m]*onehot_y[hw, s] + b1[m] )
    (the gather of sampled points is folded into a one-hot matmul; this is
     exact because onehot selects the hw = idx[s] row)
    h2T = relu(W2^T @ h1T + b2)     [512, 196]
    predT = W3^T @ h2T + b3         [2, 196]
All matmuls in bf16 with fp32 PSUM accumulation.

One-hot construction (per pair of batches): idx rows are broadcast across
partitions with a K=1 outer-product matmul (ones[1,128]^T @ idx[1,392] ->
PSUM), then compared against a per-partition iota column with is_equal.

deltaxy = float(pxs - pys) + 13 computed on-device with DVE ops.
"""
import sys
import types

import numpy as np

B, C, H, W_IMG = 256, 768, 14, 14
S = 196          # sample count == H*W
HID = 512
OUT = 2
N_CORES = 8
BPC = B // N_CORES      # batches per core
KC = C // 128           # 6 contraction chunks for layer 1
MJ = HID // 128         # 4 HID chunks
HW_CHUNKS = [(0, 128), (128, 68)]   # hw partition chunks of 196
XG = 4                  # batches per steady-state x-load group
GROUPS = [(0, 1), (1, 3)] + [(4 + 4 * i, 4) for i in range((BPC - 4) // 4)]
B2G = {}
for _gi, (_st, _ln) in enumerate(GROUPS):
    for _b in range(_st, _st + _ln):
        B2G[_b] = (_gi, _b - _st)
S2 = 2 * S              # pair width


def _install_ntff_hook():
    try:
        import antenv.axon_hooks  # noqa: F401
        return
    except ImportError:
        pass
    try:
        from trn_agent_boot.trn_boot import _ntff_profile_via_ctypes
        hook = _ntff_profile_via_ctypes("/opt/axon/libaxon_pjrt.so")
    except Exception:
        hook = None
    mod = types.ModuleType("antenv.axon_hooks")
    mod.get_axon_ntff_profile_hook = lambda: hook
    sys.modules["antenv.axon_hooks"] = mod


def _build_nc():
    from contextlib import ExitStack

    import concourse.bass as bass
    import concourse.bacc as bacc
    import concourse.mybir as mybir
    import concourse.tile as tile

    dt = mybir.dt
    f32, bf16, i32 = dt.float32, dt.bfloat16, dt.int32
    AF = mybir.ActivationFunctionType
    ALU = mybir.AluOpType

    nc = bacc.Bacc(None, target_bir_lowering=False)

    # x comes host-permuted as [KC, 128, BPC*S] so DMA segments are large
    x_t = nc.dram_tensor("x", [KC, 128, BPC * S], f32, kind="ExternalInput")
    pxs_t = nc.dram_tensor("pxs", [BPC, S2], i32, kind="ExternalInput")
    pys_t = nc.dram_tensor("pys", [BPC, S2], i32, kind="ExternalInput")
    w1_t = nc.dram_tensor("W1", [2 * C, HID], bf16, kind="ExternalInput")
    w2_t = nc.dram_tensor("W2", [HID, HID], bf16, kind="ExternalInput")
    w3_t = nc.dram_tensor("W3", [HID, OUT], bf16, kind="ExternalInput")
    b1_t = nc.dram_tensor("b1", [HID], f32, kind="ExternalInput")
    b2_t = nc.dram_tensor("b2", [HID], f32, kind="ExternalInput")
    b3_t = nc.dram_tensor("b3", [OUT], f32, kind="ExternalInput")
    iota_t = nc.dram_tensor("iota196", [S], f32, kind="ExternalInput")
    # predT: [2, BPC*S] (component-major); host transposes after gather
    pred_t = nc.dram_tensor("predT", [OUT, BPC * S], f32, kind="ExternalOutput")
    delta_t = nc.dram_tensor("deltaxy", [BPC * S, OUT], f32, kind="ExternalOutput")

    with ExitStack() as ctx:
        tc = ctx.enter_context(tile.TileContext(nc))
        wpool = ctx.enter_context(tc.tile_pool(name="w", bufs=1))
        wstage = ctx.enter_context(tc.tile_pool(name="wstage", bufs=2))
        xpool = ctx.enter_context(tc.tile_pool(name="x", bufs=2))
        xbfpool = ctx.enter_context(tc.tile_pool(name="xbf", bufs=2))
        zsb = ctx.enter_context(tc.tile_pool(name="zsb", bufs=2))
        ohpool = ctx.enter_context(tc.tile_pool(name="oh", bufs=2))
        h1pool = ctx.enter_context(tc.tile_pool(name="h1", bufs=2))
        h2pool = ctx.enter_context(tc.tile_pool(name="h2", bufs=2))
        opool = ctx.enter_context(tc.tile_pool(name="op", bufs=1))
        idxpool = ctx.enter_context(tc.tile_pool(name="idx", bufs=1))
        zps = ctx.enter_context(tc.tile_pool(name="zps", bufs=1, space="PSUM"))
        hps = ctx.enter_context(tc.tile_pool(name="hps", bufs=1, space="PSUM"))
        bcps = ctx.enter_context(tc.tile_pool(name="bcps", bufs=2, space="PSUM"))
        dram = ctx.enter_context(tc.tile_pool(name="dram", bufs=1, space="DRAM"))

        # ---------------- early: batch-0 x slab so PE can start fast --------
        xpf = xpool.tile([128, KC, 1 * S], f32, name="xs_0", tag="xs",
                         padded_shape=[128, KC, XG * S])
        nc.sync.dma_start(
            xpf[:],
            bass.AP(x_t, 0, [[BPC * S, 128], [128 * BPC * S, KC], [1, S]]),
        )
        # ---------------- index prep first (feeds the first PE one-hot) -----
        pxs_sb = idxpool.tile([BPC, S2], i32, name="pxs_sb", tag="pxs_sb")
        nc.sync.dma_start(pxs_sb[:], pxs_t[:, :])
        pys_sb = idxpool.tile([BPC, S2], i32, name="pys_sb", tag="pys_sb")
        nc.sync.dma_start(pys_sb[:], pys_t[:, :])
        iota_lo = wpool.tile([128, 1], f32, name="iota_lo", tag="iota_lo")
        nc.sync.dma_start(iota_lo[:], iota_t[0:128])
        iota_hi = wpool.tile([68, 1], f32, name="iota_hi", tag="iota_hi")
        nc.sync.dma_start(iota_hi[:], iota_t[128:S])
        ones_row = wpool.tile([1, 128], bf16, name="ones_row", tag="ones_row")
        nc.vector.memset(ones_row[:], 1.0)

        idx_rows = {}
        for nm, src in (("x", pxs_sb), ("y", pys_sb)):
            tmp = idxpool.tile([BPC, S], i32, name=f"itmp{nm}", tag=f"itmp{nm}")
            nc.vector.tensor_scalar(
                tmp[:], src[:, 0:S2:2], float(W_IMG), None, op0=ALU.mult
            )
            ib = idxpool.tile([BPC, S], bf16, name=f"ibf{nm}", tag=f"ibf{nm}")
            nc.vector.tensor_tensor(ib[:], tmp[:], src[:, 1:S2:2], ALU.add)
            d = dram.tile([BPC, S], bf16, name=f"idxd{nm}", tag=f"idxd{nm}")
            nc.sync.dma_start(d[:, :], ib[:])
            row = idxpool.tile([1, BPC * S], bf16, name=f"irow{nm}", tag=f"irow{nm}")
            nc.sync.dma_start(row[:], bass.AP(d.tensor, d[:, :].offset,
                                              [[BPC * S, 1], [1, BPC * S]]))
            idx_rows[nm] = row

        # PE warm-up: harmless matmuls so HAM reaches 8/8 before real work
        wmt = bcps.tile([128, 128], f32, name="warm", tag="bcps")
        for _ in range(24):
            nc.tensor.matmul(wmt[:], ones_row[:], ones_row[:],
                             start=True, stop=True)

        # ---------------- weights: host-cast bf16, loaded directly ----------
        w1b = []
        for k in range(2 * KC):
            wb = wpool.tile([128, HID], bf16, name=f"w1b{k}", tag=f"w1b{k}")
            nc.sync.dma_start(wb[:], w1_t[k * 128:(k + 1) * 128, :])
            w1b.append(wb)
        w2b = []
        for k in range(MJ):
            wb = wpool.tile([128, HID], bf16, name=f"w2b{k}", tag=f"w2b{k}")
            nc.sync.dma_start(wb[:], w2_t[k * 128:(k + 1) * 128, :])
            w2b.append(wb)
        w3b = []
        for k in range(MJ):
            wb = wpool.tile([128, OUT], bf16, name=f"w3b{k}", tag=f"w3b{k}")
            nc.sync.dma_start(wb[:], w3_t[k * 128:(k + 1) * 128, :])
            w3b.append(wb)
        b1c, b2c = [], []
        for j in range(MJ):
            t1 = wpool.tile([128, 1], f32, name=f"b1c{j}", tag=f"b1c{j}")
            nc.sync.dma_start(t1[:], b1_t[j * 128:(j + 1) * 128])
            b1c.append(t1)
            t2 = wpool.tile([128, 1], f32, name=f"b2c{j}", tag=f"b2c{j}")
            nc.sync.dma_start(t2[:], b2_t[j * 128:(j + 1) * 128])
            b2c.append(t2)
        b3c = wpool.tile([OUT, 1], f32, name="b3c", tag="b3c")
        nc.sync.dma_start(b3c[:], b3_t[:])

        # ---------------- deltaxy ------------------------------------------
        dsub = idxpool.tile([BPC, S2], i32, name="dsub", tag="dsub")
        nc.vector.tensor_tensor(dsub[:], pxs_sb[:], pys_sb[:], ALU.subtract)
        ddel = idxpool.tile([BPC, S2], f32, name="ddel", tag="ddel")
        nc.vector.tensor_scalar(ddel[:], dsub[:], float(H - 1), None, op0=ALU.add)
        nc.sync.dma_start(bass.AP(delta_t, 0, [[S2, BPC], [1, S2]]), ddel[:])

        pred_all = opool.tile([OUT, BPC * S], f32, name="pred_all", tag="pred_all")

        # ---------------- per-batch emission helpers ------------------------
        onehots = {}     # pair -> {(branch, ci): tile [sz, S2]}
        z_sb = {}        # b -> {(branch, ci): tile}
        h1_pair = {}     # pair -> [4 tiles [128, S2] bf16]
        xbf_g = {}       # group -> tile [128, KC, XG*S] bf16

        def emit_xgroup(g):
            st, ln = GROUPS[g]
            if g == 0:
                xs = xpf
            else:
                xs = xpool.tile([128, KC, ln * S], f32, name=f"xs_{g}", tag="xs",
                                padded_shape=[128, KC, XG * S])
                nc.sync.dma_start(
                    xs[:],
                    bass.AP(x_t, st * S,
                            [[BPC * S, 128], [128 * BPC * S, KC], [1, ln * S]]),
                )
            xb = xbfpool.tile([128, KC, ln * S], bf16, name=f"xbf_{g}", tag="xbf",
                              padded_shape=[128, KC, XG * S])
            nc.vector.tensor_copy(xb[:], xs[:])
            xbf_g[g] = xb

        def emit_onehots(pair):
            """Outer-product broadcast + is_equal one-hots for a batch pair."""
            oh = {}
            for nm, br in (("x", 0), ("y", 1)):
                bc = bcps.tile([128, S2], f32, name=f"bc{nm}_{pair}", tag="bcps")
                nc.tensor.matmul(
                    bc[:], ones_row[:],
                    idx_rows[nm][:, pair * S2:(pair + 1) * S2],
                    start=True, stop=True,
                )
                for ci, (off, sz) in enumerate(HW_CHUNKS):
                    o = ohpool.tile([sz, S2], bf16, name=f"oh{nm}{ci}_{pair}",
                                    tag=f"oh{nm}{ci}")
                    iot = (iota_lo, iota_hi)[ci]
                    nc.vector.tensor_scalar(
                        o[:], bc[0:sz, :], iot[:], None, op0=ALU.is_equal
                    )
                    oh[(br, ci)] = o
            onehots[pair] = oh

        def emit_proj(b, ci):
            """Projection matmuls for one hw chunk of batch b + PSUM->SBUF."""
            off, sz = HW_CHUNKS[ci]
            g, gi = B2G[b]
            xb = xbf_g[g]
            zt = z_sb.setdefault(b, {})
            for br in range(2):
                zp = zps.tile([sz, HID], f32, name=f"zp{br}{ci}_{b}",
                              tag=f"zp{br}")
                for k in range(KC):
                    nc.tensor.matmul(
                        zp[:],
                        xb[:, k, gi * S + off:gi * S + off + sz],
                        w1b[br * KC + k][:],
                        start=(k == 0),
                        stop=(k == KC - 1),
                    )
                zs = zsb.tile([sz, HID], bf16, name=f"zs{br}{ci}_{b}",
                              tag=f"zs{br}{ci}")
                if br == 0:
                    nc.scalar.copy(zs[:], zp[:])
                else:
                    nc.vector.tensor_copy(zs[:], zp[:])
                zt[(br, ci)] = zs

        def emit_gather(b):
            """one-hot gather matmuls + bias+relu into the pair tile."""
            pair, col = divmod(b, 2)
            if col == 0:
                h1_pair[pair] = [
                    h1pool.tile([128, S2], bf16, name=f"h1p{pair}_{j}",
                                tag=f"h1p{j}")
                    for j in range(MJ)
                ]
            zt = z_sb.pop(b)
            oh = onehots[pair]
            if col == 1:
                onehots.pop(pair)
            for j in range(MJ):
                hp = hps.tile([128, S], f32, name=f"h1ps{j}_{b}", tag=f"hps{j}")
                n_mm = 0
                for br in range(2):
                    for ci, (off, sz) in enumerate(HW_CHUNKS):
                        nc.tensor.matmul(
                            hp[:],
                            zt[(br, ci)][:, j * 128:(j + 1) * 128],
                            oh[(br, ci)][:, col * S:(col + 1) * S],
                            start=(n_mm == 0),
                            stop=(n_mm == 3),
                        )
                        n_mm += 1
                dst = h1_pair[pair][j][:, col * S:(col + 1) * S]
                if j % 2 == 0:
                    nc.scalar.activation(dst, hp[:], AF.Relu, bias=b1c[j][:])
                else:
                    nc.vector.tensor_scalar(
                        dst, hp[:], b1c[j][:], 0.0, op0=ALU.add, op1=ALU.max
                    )

        def emit_tail(pair):
            """GEMM2 + relu, GEMM3 + bias into pred_all, for a finished pair."""
            h1t = h1_pair.pop(pair)
            h2t = []
            for j in range(MJ):
                hp = hps.tile([128, S2], f32, name=f"h2ps{j}_p{pair}",
                              tag=f"hps{j}")
                for k in range(MJ):
                    nc.tensor.matmul(
                        hp[:],
                        w2b[k][:, j * 128:(j + 1) * 128],
                        h1t[k][:],
                        start=(k == 0),
                        stop=(k == MJ - 1),
                    )
                h2 = h2pool.tile([128, S2], bf16, name=f"h2sb{j}_p{pair}",
                                 tag=f"h2sb{j}")
                if j % 2 == 0:
                    nc.scalar.activation(h2[:], hp[:], AF.Relu, bias=b2c[j][:])
                else:
                    nc.vector.tensor_scalar(
                        h2[:], hp[:], b2c[j][:], 0.0, op0=ALU.add, op1=ALU.max
                    )
                h2t.append(h2)
            pp = hps.tile([OUT, S2], f32, name=f"predps_p{pair}", tag="hps0")
            for k in range(MJ):
                nc.tensor.matmul(
                    pp[:], w3b[k][:], h2t[k][:], start=(k == 0), stop=(k == MJ - 1)
                )
            nc.vector.tensor_scalar(
                pred_all[:, pair * S2:(pair + 1) * S2], pp[:], b3c[:], None,
                op0=ALU.add,
            )
            if pair % 4 == 3:
                q = pair // 4
                nc.sync.dma_start(
                    pred_t[:, q * 4 * S2:(q + 1) * 4 * S2],
                    pred_all[:, q * 4 * S2:(q + 1) * 4 * S2],
                )

        # ---------------- main loop: software-pipelined by one batch --------
        # PE order per b: [pair front: onehot bcast MMs] proj_lo(b),
        # gather(b-1), proj_hi(b), [pair tail: gemm2+gemm3]
        emit_xgroup(0)
        for b in range(BPC):
            g, gi = B2G[b]
            if gi == 0 and g + 1 < len(GROUPS):
                emit_xgroup(g + 1)
            if b % 2 == 0:
                emit_onehots(b // 2)
            emit_proj(b, 0)
            if b >= 1:
                emit_gather(b - 1)
            emit_proj(b, 1)
            if b >= 1 and (b - 1) % 2 == 1:
                emit_tail((b - 1) // 2)
        emit_gather(BPC - 1)
        emit_tail((BPC - 1) // 2)

    nc.finalize()
    return nc


_NC = None


def _get_nc():
    global _NC
    if _NC is None:
        _install_ntff_hook()
        _NC = _build_nc()
    return _NC


def _make_in_maps(inputs):
    x = np.asarray(inputs["x"], dtype=np.float32).reshape(B, C, H * W_IMG)
    pxs = np.asarray(inputs["pxs"], dtype=np.int32).reshape(B, S2)
    pys = np.asarray(inputs["pys"], dtype=np.int32).reshape(B, S2)
    import ml_dtypes
    W1 = np.asarray(np.asarray(inputs["W1"], dtype=np.float32),
                    dtype=ml_dtypes.bfloat16)
    W2 = np.asarray(np.asarray(inputs["W2"], dtype=np.float32),
                    dtype=ml_dtypes.bfloat16)
    W3 = np.asarray(np.asarray(inputs["W3"], dtype=np.float32),
                    dtype=ml_dtypes.bfloat16)
    b1 = np.asarray(inputs["b1"], dtype=np.float32)
    b2 = np.asarray(inputs["b2"], dtype=np.float32)
    b3 = np.asarray(inputs["b3"], dtype=np.float32)
    iota = np.arange(S, dtype=np.float32)
    in_maps = []
    for c in range(N_CORES):
        sl = slice(c * BPC, (c + 1) * BPC)
        # [BPC, C, S] -> [KC, 128, BPC*S] so per-partition DMA segments are
        # BPC*S contiguous floats per channel
        xc = np.ascontiguousarray(
            x[sl].reshape(BPC, KC, 128, S).transpose(1, 2, 0, 3)
        ).reshape(KC, 128, BPC * S)
        in_maps.append({
            "x": xc,
            "pxs": np.ascontiguousarray(pxs[sl]),
            "pys": np.ascontiguousarray(pys[sl]),
            "W1": W1, "W2": W2, "W3": W3,
            "b1": b1, "b2": b2, "b3": b3,
            "iota196": iota,
        })
    return in_maps


def _run(inputs, trace=False):
    from concourse.bass_utils import run_bass_kernel_spmd

    nc = _get_nc()
    in_maps = _make_in_maps(inputs)
    res = run_bass_kernel_spmd(
        nc, in_maps, core_ids=list(range(N_CORES)), trace=trace
    )
    pred = np.concatenate(
        [np.ascontiguousarray(res.results[c]["predT"].T) for c in range(N_CORES)],
        axis=0,
    )
    delta = np.concatenate(
        [res.results[c]["deltaxy"] for c in range(N_CORES)], axis=0
    )
    return (pred, delta), res


def kernel(**inputs):
    (pred, delta), _ = _run(inputs, trace=False)
    return pred, delta


# revision 6
# speedup vs baseline: 1.4280x; 1.0202x over previous
"""Trainium2 Bass kernel for nn_DenseRelativeLoc.

Data-parallel over batch: 32 images per NeuronCore x 8 cores.

Per image b (all activations kept transposed, feature-dim on partitions):
    z1a = x_flat[b]^T @ W1[:768]    [196hw, 512]   (natural layouts, no transposes)
    z1b = x_flat[b]^T @ W1[768:]    [196hw, 512]
    h1T[m, s] = relu( sum_hw z1a[hw, m]*onehot_x[hw, s]
                    + sum_hw z1b[hw, m]*onehot_y[hw, s] + b1[m] )
    (the gather of sampled points is folded into a one-hot matmul; this is
     exact because onehot selects the hw = idx[s] row)
    h2T = relu(W2^T @ h1T + b2)     [512, 196]
    predT = W3^T @ h2T + b3         [2, 196]
All matmuls in bf16 with fp32 PSUM accumulation.

One-hot construction (per pair of batches): idx rows are broadcast across
partitions with a K=1 outer-product matmul (ones[1,128]^T @ idx[1,392] ->
PSUM), then compared against a per-partition iota column with is_equal.

deltaxy = float(pxs - pys) + 13 computed on-device with DVE ops.
"""
import sys
import types

import numpy as np

B, C, H, W_IMG = 256, 768, 14, 14
S = 196          # sample count == H*W
HID = 512
OUT = 2
N_CORES = 8
BPC = B // N_CORES      # batches per core
KC = C // 128           # 6 contraction chunks for layer 1
MJ = HID // 128         # 4 HID chunks
HW_CHUNKS = [(0, 128), (128, 68)]   # hw partition chunks of 196
XG = 4                  # batches per steady-state x-load group
GROUPS = [(0, 1), (1, 3)] + [(4 + 4 * i, 4) for i in range((BPC - 4) // 4)]
B2G = {}
for _gi, (_st, _ln) in enumerate(GROUPS):
    for _b in range(_st, _st + _ln):
        B2G[_b] = (_gi, _b - _st)
S2 = 2 * S              # pair width


def _install_ntff_hook():
    try:
        import antenv.axon_hooks  # noqa: F401
        return
    except ImportError:
        pass
    try:
        from trn_agent_boot.trn_boot import _ntff_profile_via_ctypes
        hook = _ntff_profile_via_ctypes("/opt/axon/libaxon_pjrt.so")
    except Exception:
        hook = None
    mod = types.ModuleType("antenv.axon_hooks")
    mod.get_axon_ntff_profile_hook = lambda: hook
    sys.modules["antenv.axon_hooks"] = mod


def _build_nc():
    from contextlib import ExitStack

    import concourse.bass as bass
    import concourse.bacc as bacc
    import concourse.mybir as mybir
    import concourse.tile as tile

    dt = mybir.dt
    f32, bf16, i32 = dt.float32, dt.bfloat16, dt.int32
    AF = mybir.ActivationFunctionType
    ALU = mybir.AluOpType

    nc = bacc.Bacc(None, target_bir_lowering=False)

    # x comes host-permuted as [KC, 128, BPC*S] bf16 so DMA segments are
    # large and feed the matmuls directly (no on-chip staging/cast)
    x_t = nc.dram_tensor("x", [KC, 128, BPC * S], bf16, kind="ExternalInput")
    pxs_t = nc.dram_tensor("pxs", [BPC, S2], i32, kind="ExternalInput")
    pys_t = nc.dram_tensor("pys", [BPC, S2], i32, kind="ExternalInput")
    w1_t = nc.dram_tensor("W1", [2 * C, HID], bf16, kind="ExternalInput")
    w2_t = nc.dram_tensor("W2", [HID, HID], bf16, kind="ExternalInput")
    w3_t = nc.dram_tensor("W3", [HID, OUT], bf16, kind="ExternalInput")
    b1_t = nc.dram_tensor("b1", [HID], f32, kind="ExternalInput")
    b2_t = nc.dram_tensor("b2", [HID], f32, kind="ExternalInput")
    b3_t = nc.dram_tensor("b3", [OUT], f32, kind="ExternalInput")
    iota_t = nc.dram_tensor("iota196", [S], f32, kind="ExternalInput")
    # predT: [2, BPC*S] (component-major); host transposes after gather
    pred_t = nc.dram_tensor("predT", [OUT, BPC * S], f32, kind="ExternalOutput")
    delta_t = nc.dram_tensor("deltaxy", [BPC * S, OUT], f32, kind="ExternalOutput")

    with ExitStack() as ctx:
        tc = ctx.enter_context(tile.TileContext(nc))
        wpool = ctx.enter_context(tc.tile_pool(name="w", bufs=1))
        wstage = ctx.enter_context(tc.tile_pool(name="wstage", bufs=2))
        xbfpool = ctx.enter_context(tc.tile_pool(name="xbf", bufs=3))
        zsb = ctx.enter_context(tc.tile_pool(name="zsb", bufs=2))
        ohpool = ctx.enter_context(tc.tile_pool(name="oh", bufs=2))
        h1pool = ctx.enter_context(tc.tile_pool(name="h1", bufs=2))
        h2pool = ctx.enter_context(tc.tile_pool(name="h2", bufs=2))
        opool = ctx.enter_context(tc.tile_pool(name="op", bufs=1))
        idxpool = ctx.enter_context(tc.tile_pool(name="idx", bufs=1))
        zps = ctx.enter_context(tc.tile_pool(name="zps", bufs=1, space="PSUM"))
        hps = ctx.enter_context(tc.tile_pool(name="hps", bufs=1, space="PSUM"))
        bcps = ctx.enter_context(tc.tile_pool(name="bcps", bufs=2, space="PSUM"))
        dram = ctx.enter_context(tc.tile_pool(name="dram", bufs=1, space="DRAM"))

        # ---------------- early: batch-0 x slab so PE can start fast --------
        xbf_g = {}
        xb0 = xbfpool.tile([128, KC, 1 * S], bf16, name="xbf_0", tag="xbf",
                           padded_shape=[128, KC, XG * S])
        nc.sync.dma_start(
            xb0[:],
            bass.AP(x_t, 0, [[BPC * S, 128], [128 * BPC * S, KC], [1, S]]),
        )
        xbf_g[0] = xb0
        # ---------------- index prep first (feeds the first PE one-hot) -----
        pxs_sb = idxpool.tile([BPC, S2], i32, name="pxs_sb", tag="pxs_sb")
        nc.sync.dma_start(pxs_sb[:], pxs_t[:, :])
        pys_sb = idxpool.tile([BPC, S2], i32, name="pys_sb", tag="pys_sb")
        nc.sync.dma_start(pys_sb[:], pys_t[:, :])
        iota_lo = wpool.tile([128, 1], f32, name="iota_lo", tag="iota_lo")
        nc.sync.dma_start(iota_lo[:], iota_t[0:128])
        iota_hi = wpool.tile([68, 1], f32, name="iota_hi", tag="iota_hi")
        nc.sync.dma_start(iota_hi[:], iota_t[128:S])
        ones_row = wpool.tile([1, 128], bf16, name="ones_row", tag="ones_row")
        nc.vector.memset(ones_row[:], 1.0)

        idx_rows = {}
        for nm, src in (("x", pxs_sb), ("y", pys_sb)):
            tmp = idxpool.tile([BPC, S], i32, name=f"itmp{nm}", tag=f"itmp{nm}")
            nc.vector.tensor_scalar(
                tmp[:], src[:, 0:S2:2], float(W_IMG), None, op0=ALU.mult
            )
            ib = idxpool.tile([BPC, S], bf16, name=f"ibf{nm}", tag=f"ibf{nm}")
            nc.vector.tensor_tensor(ib[:], tmp[:], src[:, 1:S2:2], ALU.add)
            d = dram.tile([BPC, S], bf16, name=f"idxd{nm}", tag=f"idxd{nm}")
            nc.sync.dma_start(d[:, :], ib[:])
            row = idxpool.tile([1, BPC * S], bf16, name=f"irow{nm}", tag=f"irow{nm}")
            nc.sync.dma_start(row[:], bass.AP(d.tensor, d[:, :].offset,
                                              [[BPC * S, 1], [1, BPC * S]]))
            idx_rows[nm] = row

        # PE warm-up: harmless matmuls so HAM reaches 8/8 before real work
        wmt = bcps.tile([128, 128], f32, name="warm", tag="bcps")
        for _ in range(24):
            nc.tensor.matmul(wmt[:], ones_row[:], ones_row[:],
                             start=True, stop=True)

        # ---------------- weights: host-cast bf16, loaded directly ----------
        w1b = []
        for k in range(2 * KC):
            wb = wpool.tile([128, HID], bf16, name=f"w1b{k}", tag=f"w1b{k}")
            nc.sync.dma_start(wb[:], w1_t[k * 128:(k + 1) * 128, :])
            w1b.append(wb)
        w2b = []
        for k in range(MJ):
            wb = wpool.tile([128, HID], bf16, name=f"w2b{k}", tag=f"w2b{k}")
            nc.sync.dma_start(wb[:], w2_t[k * 128:(k + 1) * 128, :])
            w2b.append(wb)
        w3b = []
        for k in range(MJ):
            wb = wpool.tile([128, OUT], bf16, name=f"w3b{k}", tag=f"w3b{k}")
            nc.sync.dma_start(wb[:], w3_t[k * 128:(k + 1) * 128, :])
            w3b.append(wb)
        b1c, b2c = [], []
        for j in range(MJ):
            t1 = wpool.tile([128, 1], f32, name=f"b1c{j}", tag=f"b1c{j}")
            nc.sync.dma_start(t1[:], b1_t[j * 128:(j + 1) * 128])
            b1c.append(t1)
            t2 = wpool.tile([128, 1], f32, name=f"b2c{j}", tag=f"b2c{j}")
            nc.sync.dma_start(t2[:], b2_t[j * 128:(j + 1) * 128])
            b2c.append(t2)
        b3c = wpool.tile([OUT, 1], f32, name="b3c", tag="b3c")
        nc.sync.dma_start(b3c[:], b3_t[:])

        # ---------------- deltaxy ------------------------------------------
        dsub = idxpool.tile([BPC, S2], i32, name="dsub", tag="dsub")
        nc.vector.tensor_tensor(dsub[:], pxs_sb[:], pys_sb[:], ALU.subtract)
        ddel = idxpool.tile([BPC, S2], f32, name="ddel", tag="ddel")
        nc.vector.tensor_scalar(ddel[:], dsub[:], float(H - 1), None, op0=ALU.add)
        nc.sync.dma_start(bass.AP(delta_t, 0, [[S2, BPC], [1, S2]]), ddel[:])

        pred_all = opool.tile([OUT, BPC * S], f32, name="pred_all", tag="pred_all")

        # ---------------- per-batch emission helpers ------------------------
        onehots = {}     # pair -> {(branch, ci): tile [sz, S2]}
        z_sb = {}        # b -> {(branch, ci): tile}
        h1_pair = {}     # pair -> [4 tiles [128, S2] bf16]

        def emit_xgroup(g):
            if g in xbf_g:
                return
            st, ln = GROUPS[g]
            xb = xbfpool.tile([128, KC, ln * S], bf16, name=f"xbf_{g}", tag="xbf",
                              padded_shape=[128, KC, XG * S])
            nc.sync.dma_start(
                xb[:],
                bass.AP(x_t, st * S,
                        [[BPC * S, 128], [128 * BPC * S, KC], [1, ln * S]]),
            )
            xbf_g[g] = xb

        def emit_onehots(pair):
            """Outer-product broadcast + is_equal one-hots for a batch pair."""
            oh = {}
            for nm, br in (("x", 0), ("y", 1)):
                bc = bcps.tile([128, S2], f32, name=f"bc{nm}_{pair}", tag="bcps")
                nc.tensor.matmul(
                    bc[:], ones_row[:],
                    idx_rows[nm][:, pair * S2:(pair + 1) * S2],
                    start=True, stop=True,
                )
                for ci, (off, sz) in enumerate(HW_CHUNKS):
                    o = ohpool.tile([sz, S2], bf16, name=f"oh{nm}{ci}_{pair}",
                                    tag=f"oh{nm}{ci}")
                    iot = (iota_lo, iota_hi)[ci]
                    nc.vector.tensor_scalar(
                        o[:], bc[0:sz, :], iot[:], None, op0=ALU.is_equal
                    )
                    oh[(br, ci)] = o
            onehots[pair] = oh

        def emit_proj(b, ci):
            """Projection matmuls for one hw chunk of batch b + PSUM->SBUF."""
            off, sz = HW_CHUNKS[ci]
            g, gi = B2G[b]
            xb = xbf_g[g]
            zt = z_sb.setdefault(b, {})
            for br in range(2):
                zp = zps.tile([sz, HID], f32, name=f"zp{br}{ci}_{b}",
                              tag=f"zp{br}")
                for k in range(KC):
                    nc.tensor.matmul(
                        zp[:],
                        xb[:, k, gi * S + off:gi * S + off + sz],
                        w1b[br * KC + k][:],
                        start=(k == 0),
                        stop=(k == KC - 1),
                    )
                zs = zsb.tile([sz, HID], bf16, name=f"zs{br}{ci}_{b}",
                              tag=f"zs{br}{ci}")
                if br == 0:
                    nc.scalar.copy(zs[:], zp[:])
                else:
                    nc.vector.tensor_copy(zs[:], zp[:])
                zt[(br, ci)] = zs

        def emit_gather(b):
            """one-hot gather matmuls + bias+relu into the pair tile."""
            pair, col = divmod(b, 2)
            if col == 0:
                h1_pair[pair] = [
                    h1pool.tile([128, S2], bf16, name=f"h1p{pair}_{j}",
                                tag=f"h1p{j}")
                    for j in range(MJ)
                ]
            zt = z_sb.pop(b)
            oh = onehots[pair]
            if col == 1:
                onehots.pop(pair)
            for j in range(MJ):
                hp = hps.tile([128, S], f32, name=f"h1ps{j}_{b}", tag=f"hps{j}")
                n_mm = 0
                for br in range(2):
                    for ci, (off, sz) in enumerate(HW_CHUNKS):
                        nc.tensor.matmul(
                            hp[:],
                            zt[(br, ci)][:, j * 128:(j + 1) * 128],
                            oh[(br, ci)][:, col * S:(col + 1) * S],
                            start=(n_mm == 0),
                            stop=(n_mm == 3),
                        )
                        n_mm += 1
                dst = h1_pair[pair][j][:, col * S:(col + 1) * S]
                if j % 2 == 0:
                    nc.scalar.activation(dst, hp[:], AF.Relu, bias=b1c[j][:])
                else:
                    nc.vector.tensor_scalar(
                        dst, hp[:], b1c[j][:], 0.0, op0=ALU.add, op1=ALU.max
                    )

        def emit_tail(pair):
            """GEMM2 + relu, GEMM3 + bias into pred_all, for a finished pair."""
            h1t = h1_pair.pop(pair)
            h2t = []
            for j in range(MJ):
                hp = hps.tile([128, S2], f32, name=f"h2ps{j}_p{pair}",
                              tag=f"hps{j}")
                for k in range(MJ):
                    nc.tensor.matmul(
                        hp[:],
                        w2b[k][:, j * 128:(j + 1) * 128],
                        h1t[k][:],
                        start=(k == 0),
                        stop=(k == MJ - 1),
                    )
                h2 = h2pool.tile([128, S2], bf16, name=f"h2sb{j}_p{pair}",
                                 tag=f"h2sb{j}")
                if j % 2 == 0:
                    nc.scalar.activation(h2[:], hp[:], AF.Relu, bias=b2c[j][:])
                else:
                    nc.vector.tensor_scalar(
                        h2[:], hp[:], b2c[j][:], 0.0, op0=ALU.add, op1=ALU.max
                    )
                h2t.append(h2)
            pp = hps.tile([OUT, S2], f32, name=f"predps_p{pair}", tag="hps0")
            for k in range(MJ):
                nc.tensor.matmul(
                    pp[:], w3b[k][:], h2t[k][:], start=(k == 0), stop=(k == MJ - 1)
                )
            nc.vector.tensor_scalar(
                pred_all[:, pair * S2:(pair + 1) * S2], pp[:], b3c[:], None,
                op0=ALU.add,
            )
            if pair % 4 == 3:
                q = pair // 4
                nc.sync.dma_start(
                    pred_t[:, q * 4 * S2:(q + 1) * 4 * S2],
                    pred_all[:, q * 4 * S2:(q + 1) * 4 * S2],
                )

        # ---------------- main loop: software-pipelined by one batch --------
        # PE order per b: [pair front: onehot bcast MMs] proj_lo(b),
        # gather(b-1), proj_hi(b), [pair tail: gemm2+gemm3]
        emit_xgroup(0)
        for b in range(BPC):
            g, gi = B2G[b]
            if gi == 0 and g + 1 < len(GROUPS):
                emit_xgroup(g + 1)
            if b % 2 == 0:
                emit_onehots(b // 2)
            emit_proj(b, 0)
            if b >= 1:
                emit_gather(b - 1)
            emit_proj(b, 1)
            if b >= 1 and (b - 1) % 2 == 1:
                emit_tail((b - 1) // 2)
        emit_gather(BPC - 1)
        emit_tail((BPC - 1) // 2)

    nc.finalize()
    return nc


_NC = None


def _get_nc():
    global _NC
    if _NC is None:
        _install_ntff_hook()
        _NC = _build_nc()
    return _NC


def _make_in_maps(inputs):
    import ml_dtypes
    x = np.asarray(inputs["x"], dtype=np.float32).reshape(B, C, H * W_IMG)
    x = x.astype(ml_dtypes.bfloat16)
    pxs = np.asarray(inputs["pxs"], dtype=np.int32).reshape(B, S2)
    pys = np.asarray(inputs["pys"], dtype=np.int32).reshape(B, S2)
    import ml_dtypes
    W1 = np.asarray(np.asarray(inputs["W1"], dtype=np.float32),
                    dtype=ml_dtypes.bfloat16)
    W2 = np.asarray(np.asarray(inputs["W2"], dtype=np.float32),
                    dtype=ml_dtypes.bfloat16)
    W3 = np.asarray(np.asarray(inputs["W3"], dtype=np.float32),
                    dtype=ml_dtypes.bfloat16)
    b1 = np.asarray(inputs["b1"], dtype=np.float32)
    b2 = np.asarray(inputs["b2"], dtype=np.float32)
    b3 = np.asarray(inputs["b3"], dtype=np.float32)
    iota = np.arange(S, dtype=np.float32)
    in_maps = []
    for c in range(N_CORES):
        sl = slice(c * BPC, (c + 1) * BPC)
        # [BPC, C, S] -> [KC, 128, BPC*S] so per-partition DMA segments are
        # BPC*S contiguous floats per channel
        xc = np.ascontiguousarray(
            x[sl].reshape(BPC, KC, 128, S).transpose(1, 2, 0, 3)
        ).reshape(KC, 128, BPC * S)
        in_maps.append({
            "x": xc,
            "pxs": np.ascontiguousarray(pxs[sl]),
            "pys": np.ascontiguousarray(pys[sl]),
            "W1": W1, "W2": W2, "W3": W3,
            "b1": b1, "b2": b2, "b3": b3,
            "iota196": iota,
        })
    return in_maps


def _run(inputs, trace=False):
    from concourse.bass_utils import run_bass_kernel_spmd

    nc = _get_nc()
    in_maps = _make_in_maps(inputs)
    res = run_bass_kernel_spmd(
        nc, in_maps, core_ids=list(range(N_CORES)), trace=trace
    )
    pred = np.concatenate(
        [np.ascontiguousarray(res.results[c]["predT"].T) for c in range(N_CORES)],
        axis=0,
    )
    delta = np.concatenate(
        [res.results[c]["deltaxy"] for c in range(N_CORES)], axis=0
    )
    return (pred, delta), res


def kernel(**inputs):
    (pred, delta), _ = _run(inputs, trace=False)
    return pred, delta
